# revision 11
# baseline (speedup 1.0000x reference)
"""Causal multi-head attention (B=2, L=2048, D=2048, H=32) on 8 trn2 NeuronCores.

Sharding: data-parallel over batch (2 groups of 4 cores) x tensor-parallel over
heads (8 heads per core). Host pre-transposes x and pre-shards/scales weights;
host sums the 4 tensor-parallel partials per batch (fp32).

v3 design:
  - fp16 operands (bf16 for p/vh so exp of unmasked garbage can't overflow
    16-bit range); fp32 PSUM accumulation; fp16 output partials.
  - Fully interleaved token-tile pipeline: for each 512-token tile n, emit the
    q/k/v projections for tile n and then SDPA for q-block n (causality means
    its whole k-range is already projected).  ACT's exp stream overlaps the
    next tile's projection matmuls, so the PE never waits out the softmax.
  - x-tile DMAs emitted one section early (prefetch during SDPA).
  - S matmuls run as head PAIRS on row-tiles (0,0)/(64,0) (K=64 concurrent);
    exp batched over [128,2,512] PSUM double-tiles; PV restricted to the
    causally valid columns; lag-2 software pipeline S->exp->PV.
  - softmax denominators: per-block rows collected at partition 64, PE-gathered
    onto partitions, ONE batched reciprocal per q-section [128,32], PE
    broadcast back; normalization deferred one section (overlaps projections).
  - ACT is exp-only during the pipeline; projection PSUM copies + triangle
    masking on DVE; outproj copies split DVE/ACT.
"""

import sys

sys.path.insert(0, "/opt/trn_rl_repo")

import numpy as np

import concourse.bass as bass
import concourse.tile as tile
from concourse import bacc, mybir
from concourse.bass_utils import run_bass_kernel_spmd


def _ensure_ntff_hook():
    """The agent image's antenv package lacks axon_hooks, which makes
    run_bass_kernel_spmd(trace=True) crash on import. Provide the module and
    register the ctypes-based NTFF profiling hook (degrades silently)."""
    try:
        import types

        import antenv

        if "antenv.axon_hooks" not in sys.modules:
            m = types.ModuleType("antenv.axon_hooks")
            state = {"hook": None}
            m.set_axon_ntff_profile_hook = lambda h: state.__setitem__("hook", h)
            m.get_axon_ntff_profile_hook = lambda: state["hook"]
            sys.modules["antenv.axon_hooks"] = m
            antenv.axon_hooks = m
        from antenv.axon_hooks import (
            get_axon_ntff_profile_hook,
            set_axon_ntff_profile_hook,
        )

        if get_axon_ntff_profile_hook() is None:
            from trn_agent_boot.trn_boot import _ntff_profile_via_ctypes

            set_axon_ntff_profile_hook(
                _ntff_profile_via_ctypes("/opt/axon/libaxon_pjrt.so")
            )
    except Exception:
        pass


_ensure_ntff_hook()

F32 = mybir.dt.float32
F16 = mybir.dt.float16
BF16 = mybir.dt.bfloat16

B, L, D, H = 2, 2048, 2048, 32
HD = 64          # head dim
N_CORES = 8
TP = 4           # tensor-parallel width (heads split 4 ways)
HPC = H // TP    # heads per core = 8
NHP = HPC // 2   # head pairs per core = 4
DH = HPC * HD    # per-core projected width = 512
SCALE = float(HD) ** -0.5

QB = 512         # query-block width for SDPA
KC = D // 128    # contraction chunks for the projections = 16
NT = L // QB     # 512-token tiles = 4
TC = L // 128    # 128-token chunks = 16
KPQ = QB // 128  # k-chunks per q-block = 4


def _emit(nc):
    xq = nc.dram_tensor("xq", [D, L], F16, kind="ExternalInput")
    xk = nc.dram_tensor("xk", [D, L], F16, kind="ExternalInput")
    xv = nc.dram_tensor("xv", [D, L], F16, kind="ExternalInput")
    wq = nc.dram_tensor("wq", [D, DH], F16, kind="ExternalInput")
    wk = nc.dram_tensor("wk", [D, DH], F16, kind="ExternalInput")
    wv = nc.dram_tensor("wv", [D, DH], F16, kind="ExternalInput")
    wo = nc.dram_tensor("wo", [DH, D], F16, kind="ExternalInput")
    # konst [128, 256]: cols 0:128 = triu ones (f16), cols 128:256 = identity
    konst = nc.dram_tensor("konst", [128, 256], F16, kind="ExternalInput")
    outT = nc.dram_tensor("outT", [D, L], F16, kind="ExternalOutput")

    EXP = mybir.ActivationFunctionType.Exp

    with tile.TileContext(nc) as tc:
        from contextlib import ExitStack

        with ExitStack() as st:
            constp = st.enter_context(tc.tile_pool(name="const", bufs=1))
            ksb = constp.tile([128, 256], F16)
            nc.sync.dma_start(ksb[:], konst[:])
            ident_sb = ksb[:, 128:256]      # identity, f16
            ones64 = constp.tile([128, 64], F16)
            nc.vector.memset(ones64[:], 1.0)
            tri_bf = constp.tile([128, 128], BF16)
            nc.vector.tensor_copy(tri_bf[:], ksb[:, 0:128])

            actp = st.enter_context(tc.tile_pool(name="acts", bufs=1))
            qhT = actp.tile([128, NHP, L], F16)
            khT = actp.tile([128, NHP, L], F16)
            # vh: per 128-token chunk, 8 heads x (64 v-dims + ones col), bf16
            vh = actp.tile([128, TC, HPC * (HD + 1)], BF16)
            # whole tile starts at 1.0; projection copies overwrite the data
            # columns, leaving the 65th (denominator) column at 1.0
            nc.vector.memset(vh[:], 1.0)
            oT = actp.tile([128, NHP, L], F16)       # normalized per-head out
            # merged head-B staging + denominator rows, [65, qb%2, jl, 512]:
            # rows 0:64 of jl=2hp+1 hold head-B unnormalized o; row 64 holds
            # the denominator row for every block-head jl = hp*2 + h
            obden = actp.tile([65, 2, 8, QB], F16)

            # ---- persistent weights + per-section x tiles ----
            wqp = st.enter_context(tc.tile_pool(name="wqp", bufs=1))
            wkp = st.enter_context(tc.tile_pool(name="wkp", bufs=1))
            wvp = st.enter_context(tc.tile_pool(name="wvp", bufs=1))
            xqp = st.enter_context(tc.tile_pool(name="xqp", bufs=1))
            xkp = st.enter_context(tc.tile_pool(name="xkp", bufs=1))
            xvp = st.enter_context(tc.tile_pool(name="xvp", bufs=1))
            wq_sb = wqp.tile([128, KC, DH], F16)
            wk_sb = wkp.tile([128, KC, DH], F16)
            wv_sb = wvp.tile([128, KC, DH], F16)
            for w_sb, wdram in ((wq_sb, wq), (wk_sb, wk), (wv_sb, wv)):
                nc.sync.dma_start(
                    w_sb[:], wdram[:].rearrange("(kc p) m -> p kc m", p=128)
                )

            ppool = st.enter_context(tc.tile_pool(name="pp", bufs=4))
            projp = st.enter_context(
                tc.tile_pool(name="projp", bufs=2, space="PSUM")
            )
            spool = st.enter_context(
                tc.tile_pool(name="sps", bufs=2, space="PSUM")
            )
            opool = st.enter_context(
                tc.tile_pool(name="ops", bufs=2, space="PSUM")
            )
            rrpool = st.enter_context(tc.tile_pool(name="rrp", bufs=4))
            drpool = st.enter_context(tc.tile_pool(name="drp", bufs=2))
            wop = st.enter_context(tc.tile_pool(name="wop", bufs=1))
            osbp = st.enter_context(tc.tile_pool(name="osbp", bufs=4))
            wo_sb = wop.tile([128, NHP, D], F16)
            nc.sync.dma_start(
                wo_sb[:], wo[:].rearrange("(kc p) m -> p kc m", p=128)
            )

            def emit_outproj(qb):
                # output projection for query columns qb*QB:(qb+1)*QB
                for m in range(D // 128):
                    pt = projp.tile([128, QB], F32, tag="ps", name="pt")
                    for kc2 in range(NHP):
                        nc.tensor.matmul(
                            pt[:],
                            wo_sb[:, kc2, m * 128 : (m + 1) * 128],
                            oT[:, kc2, qb * QB : (qb + 1) * QB],
                            start=(kc2 == 0),
                            stop=(kc2 == NHP - 1),
                        )
                    osb = osbp.tile([128, QB], F16, tag="ot", name="osb")
                    if m % 2 == 0:
                        nc.vector.tensor_copy(osb[:], pt[:])
                    else:
                        nc.scalar.copy(osb[:], pt[:])
                    nc.sync.dma_start(
                        outT[m * 128 : (m + 1) * 128, qb * QB : (qb + 1) * QB],
                        osb[:],
                    )

            def emit_x_dma(n):
                tiles = {}
                for key, xp, xdram in (
                    ("q", xqp, xq), ("k", xkp, xk), ("v", xvp, xv)
                ):
                    x_sb = xp.tile([128, KC, QB], F16, tag="x", name=f"x{key}")
                    nc.sync.dma_start(
                        x_sb[:],
                        xdram[:, n * QB : (n + 1) * QB].rearrange(
                            "(kc p) t -> p kc t", p=128
                        ),
                    )
                    tiles[key] = x_sb
                return tiles

            def emit_proj(n, xt):
                # q/k projections: head dims on partitions
                for w_sb, dst, key in (
                    (wq_sb, qhT, "q"), (wk_sb, khT, "k")
                ):
                    for m in range(NHP):
                        ps = projp.tile([128, QB], F32, tag="ps", name="ps")
                        for kc in range(KC):
                            nc.tensor.matmul(
                                ps[:],
                                w_sb[:, kc, m * 128 : (m + 1) * 128],
                                xt[key][:, kc, :],
                                start=(kc == 0),
                                stop=(kc == KC - 1),
                            )
                        nc.vector.tensor_copy(
                            dst[:, m, n * QB : (n + 1) * QB], ps[:]
                        )
                # v projection: tokens on partitions
                for tt in range(KPQ):
                    ps = projp.tile([128, QB], F32, tag="ps", name="ps")
                    for kc in range(KC):
                        nc.tensor.matmul(
                            ps[:, 0:DH],
                            xt["v"][:, kc, tt * 128 : (tt + 1) * 128],
                            wv_sb[:, kc, :],
                            start=(kc == 0),
                            stop=(kc == KC - 1),
                        )
                    tci = n * KPQ + tt
                    vdst = vh[:, tci, :].rearrange("p (h c) -> p h c", c=HD + 1)
                    nc.vector.tensor_copy(
                        vdst[:, :, 0:HD],
                        ps[:, 0:DH].rearrange("p (h d) -> p h d", d=HD),
                    )

            # ---- SDPA section for q-block qb: blocks (hp, qb), lag-2 ----
            def emit_sdpa_section(qb):
                kcnt = (qb + 1) * KPQ
                q0 = qb * QB
                steps = [(hp, kc) for hp in range(NHP) for kc in range(kcnt)]
                state = {}

                def emit_s(i):
                    hp, kc = steps[i]
                    if hp not in state:
                        state[hp] = {"p": {}, "o": None}
                    stt = state[hp]
                    dj = kc - qb * KPQ
                    c0 = 128 * dj if dj > 0 else 0
                    spair = spool.tile([128, 2, QB], F32, tag="s", name="spair")
                    nc.tensor.matmul(
                        spair[:, 0, c0:QB],
                        khT[0:64, hp, kc * 128 : (kc + 1) * 128],
                        qhT[0:64, hp, q0 + c0 : q0 + QB],
                        start=True,
                        stop=True,
                    )
                    nc.tensor.matmul(
                        spair[:, 1, c0:QB],
                        khT[64:128, hp, kc * 128 : (kc + 1) * 128],
                        qhT[64:128, hp, q0 + c0 : q0 + QB],
                        start=True,
                        stop=True,
                    )
                    ppair = ppool.tile([128, 2, QB], BF16, tag="p", name="ppair")
                    nc.scalar.activation(
                        ppair[:, :, c0:QB], spair[:, :, c0:QB], EXP
                    )
                    if dj >= 0:
                        for h in range(2):
                            nc.vector.tensor_mul(
                                ppair[:, h, c0 : c0 + 128],
                                ppair[:, h, c0 : c0 + 128],
                                tri_bf[:],
                            )
                    stt["p"][kc] = ppair

                def emit_pv(i):
                    hp, kc = steps[i]
                    stt = state[hp]
                    dj = kc - qb * KPQ
                    c0 = 128 * dj if dj > 0 else 0
                    if kc == 0:
                        stt["o"] = (
                            opool.tile([65, QB], F32, tag="o", name="o_a"),
                            opool.tile([65, QB], F32, tag="o", name="o_b"),
                        )
                    ppair = stt["p"].pop(kc)
                    for h in range(2):
                        nc.tensor.matmul(
                            stt["o"][h][:, c0:QB],
                            vh[:, kc, (2 * hp + h) * (HD + 1) : (2 * hp + h + 1) * (HD + 1)],
                            ppair[:, h, c0:QB],
                            start=(kc == 0),
                            stop=(kc == kcnt - 1),
                        )
                    if kc == kcnt - 1:
                        # unnormalized o -> SBUF; denominator rows -> obden
                        stt = state.pop(hp)
                        oA, oB = stt["o"]
                        nc.vector.tensor_copy(
                            oT[0:64, hp, q0 : q0 + QB], oA[0:64, :]
                        )
                        nc.vector.tensor_copy(
                            obden[64:65, qb % 2, hp * 2, :], oA[64:65, :]
                        )
                        nc.vector.tensor_copy(
                            obden[0:65, qb % 2, hp * 2 + 1, :], oB[0:65, :]
                        )

                LAG = 2
                for i in range(len(steps) + LAG):
                    if i < len(steps):
                        emit_s(i)
                    if i - LAG >= 0:
                        emit_pv(i - LAG)

            # ---- deferred normalization for section qb ----
            def emit_norm(qb):
                q0 = qb * QB
                den_t = projp.tile([128, QB], F32, tag="ps", name="den_t")
                for jl in range(8):              # jl = hp*2 + head
                    for c in range(4):
                        nc.tensor.matmul(
                            den_t[:, jl * 4 + c : jl * 4 + c + 1],
                            obden[64:65, qb % 2, jl, c * 128 : (c + 1) * 128],
                            ones64[64:65, 0:1],
                            start=(jl == 0 and c == 0),
                            stop=(jl == 7 and c == 3),
                            skip_group_check=True,
                        )
                den_rt = drpool.tile([128, 32], F32, tag="dr", name="den_rt")
                nc.vector.reciprocal(den_rt[:], den_t[:, 0:32])
                for jl in range(8):
                    hp, h = jl // 2, jl % 2
                    bc = projp.tile([128, QB], F32, tag="ps", name="bc")
                    for c in range(4):
                        rrep = rrpool.tile([128, 64], F16, tag="rr", name="rrep")
                        nc.vector.tensor_scalar_mul(
                            rrep[:],
                            ones64[:],
                            den_rt[:, jl * 4 + c : jl * 4 + c + 1],
                        )
                        nc.tensor.matmul(
                            bc[0:64, c * 128 : (c + 1) * 128],
                            rrep[:],
                            ident_sb[:],
                            start=(c == 0),
                            stop=(c == 3),
                            skip_group_check=True,
                        )
                    if h == 0:
                        dst = oT[0:64, hp, q0 : q0 + QB]
                        nc.vector.tensor_mul(dst, dst, bc[0:64, :])
                    else:
                        dst = obden[0:64, qb % 2, hp * 2 + 1, :]
                        nc.vector.tensor_mul(dst, dst, bc[0:64, :])
                        nc.sync.dma_start(
                            oT[64:128, hp, q0 : q0 + QB], dst
                        )

            # ---- main interleaved pipeline ----
            xt = emit_x_dma(0)
            for n in range(NT):
                emit_proj(n, xt)
                if n + 1 < NT:
                    xt = emit_x_dma(n + 1)
                if n >= 1:
                    emit_norm(n - 1)
                emit_sdpa_section(n)
                if n >= 1:
                    emit_outproj(n - 1)
            emit_norm(NT - 1)
            emit_outproj(NT - 1)

    return nc


def build():
    nc = bacc.Bacc("TRN2", target_bir_lowering=False, debug=False)
    _emit(nc)
    nc.compile()
    return nc


_NC_CACHE = {}


def _get_nc():
    if "nc" not in _NC_CACHE:
        _NC_CACHE["nc"] = build()
    return _NC_CACHE["nc"]


def make_in_maps(q, k, v, Wq, Wk, Wv, Wo):
    konst_m = np.zeros((128, 256), dtype=np.float16)
    konst_m[:, 0:128] = np.triu(np.ones((128, 128), dtype=np.float16))
    konst_m[:, 128:256] = np.eye(128, dtype=np.float16)
    qT = [np.ascontiguousarray(q[b].T).astype(np.float16) for b in range(B)]
    kT = [np.ascontiguousarray(k[b].T).astype(np.float16) for b in range(B)]
    vT = [np.ascontiguousarray(v[b].T).astype(np.float16) for b in range(B)]
    wq_s, wk_s, wv_s, wo_s = [], [], [], []
    for tp in range(TP):
        rows = slice(tp * DH, (tp + 1) * DH)
        wq_s.append(np.ascontiguousarray(Wq[rows].T * SCALE).astype(np.float16))
        wk_s.append(np.ascontiguousarray(Wk[rows].T).astype(np.float16))
        wv_s.append(np.ascontiguousarray(Wv[rows].T).astype(np.float16))
        wo_s.append(np.ascontiguousarray(Wo[:, rows].T).astype(np.float16))
    in_maps = []
    for c in range(N_CORES):
        b, tp = c // TP, c % TP
        in_maps.append(
            {
                "xq": qT[b],
                "xk": kT[b],
                "xv": vT[b],
                "wq": wq_s[tp],
                "wk": wk_s[tp],
                "wv": wv_s[tp],
                "wo": wo_s[tp],
                "konst": konst_m,
            }
        )
    return in_maps


def kernel(q, k, v, Wq, Wk, Wv, Wo, mask=None, trace=False):
    q = np.asarray(q, dtype=np.float32)
    k = np.asarray(k, dtype=np.float32)
    v = np.asarray(v, dtype=np.float32)
    nc = _get_nc()
    in_maps = make_in_maps(
        q, k, v,
        np.asarray(Wq, np.float32), np.asarray(Wk, np.float32),
        np.asarray(Wv, np.float32), np.asarray(Wo, np.float32),
    )
    res = run_bass_kernel_spmd(
        nc, in_maps, core_ids=list(range(N_CORES)), trace=trace
    )
    out = np.zeros((B, L, D), dtype=np.float32)
    for c in range(N_CORES):
        out[c // TP] += res.results[c]["outT"].T.astype(np.float32)
    if trace:
        return out, res
    return out


# revision 12
# speedup vs baseline: 1.0162x; 1.0162x over previous
"""Causal multi-head attention (B=2, L=2048, D=2048, H=32) on 8 trn2 NeuronCores.

Sharding: data-parallel over batch (2 groups of 4 cores) x tensor-parallel over
heads (8 heads per core). Host pre-transposes x and pre-shards/scales weights;
host sums the 4 tensor-parallel partials per batch (fp32).

v3 design:
  - fp16 operands (bf16 for p/vh so exp of unmasked garbage can't overflow
    16-bit range); fp32 PSUM accumulation; fp16 output partials.
  - Fully interleaved token-tile pipeline: for each 512-token tile n, emit the
    q/k/v projections for tile n and then SDPA for q-block n (causality means
    its whole k-range is already projected).  ACT's exp stream overlaps the
    next tile's projection matmuls, so the PE never waits out the softmax.
  - x-tile DMAs emitted one section early (prefetch during SDPA).
  - S matmuls run as head PAIRS on row-tiles (0,0)/(64,0) (K=64 concurrent);
    exp batched over [128,2,512] PSUM double-tiles; PV restricted to the
    causally valid columns; lag-2 software pipeline S->exp->PV.
  - softmax denominators: per-block rows collected at partition 64, PE-gathered
    onto partitions, ONE batched reciprocal per q-section [128,32], PE
    broadcast back; normalization deferred one section (overlaps projections).
  - ACT is exp-only during the pipeline; projection PSUM copies + triangle
    masking on DVE; outproj copies split DVE/ACT.
"""

import sys

sys.path.insert(0, "/opt/trn_rl_repo")

import numpy as np

import concourse.bass as bass
import concourse.tile as tile
from concourse import bacc, mybir
from concourse.bass_utils import run_bass_kernel_spmd


def _ensure_ntff_hook():
    """The agent image's antenv package lacks axon_hooks, which makes
    run_bass_kernel_spmd(trace=True) crash on import. Provide the module and
    register the ctypes-based NTFF profiling hook (degrades silently)."""
    try:
        import types

        import antenv

        if "antenv.axon_hooks" not in sys.modules:
            m = types.ModuleType("antenv.axon_hooks")
            state = {"hook": None}
            m.set_axon_ntff_profile_hook = lambda h: state.__setitem__("hook", h)
            m.get_axon_ntff_profile_hook = lambda: state["hook"]
            sys.modules["antenv.axon_hooks"] = m
            antenv.axon_hooks = m
        from antenv.axon_hooks import (
            get_axon_ntff_profile_hook,
            set_axon_ntff_profile_hook,
        )

        if get_axon_ntff_profile_hook() is None:
            from trn_agent_boot.trn_boot import _ntff_profile_via_ctypes

            set_axon_ntff_profile_hook(
                _ntff_profile_via_ctypes("/opt/axon/libaxon_pjrt.so")
            )
    except Exception:
        pass


_ensure_ntff_hook()

F32 = mybir.dt.float32
F16 = mybir.dt.float16
BF16 = mybir.dt.bfloat16

B, L, D, H = 2, 2048, 2048, 32
HD = 64          # head dim
N_CORES = 8
TP = 4           # tensor-parallel width (heads split 4 ways)
HPC = H // TP    # heads per core = 8
NHP = HPC // 2   # head pairs per core = 4
DH = HPC * HD    # per-core projected width = 512
SCALE = float(HD) ** -0.5

QB = 512         # query-block width for SDPA
KC = D // 128    # contraction chunks for the projections = 16
NT = L // QB     # 512-token tiles = 4
TC = L // 128    # 128-token chunks = 16
KPQ = QB // 128  # k-chunks per q-block = 4


def _emit(nc):
    xq = nc.dram_tensor("xq", [D, L], F16, kind="ExternalInput")
    xk = nc.dram_tensor("xk", [D, L], F16, kind="ExternalInput")
    xv = nc.dram_tensor("xv", [D, L], F16, kind="ExternalInput")
    wq = nc.dram_tensor("wq", [D, DH], F16, kind="ExternalInput")
    wk = nc.dram_tensor("wk", [D, DH], F16, kind="ExternalInput")
    wv = nc.dram_tensor("wv", [D, DH], F16, kind="ExternalInput")
    wo = nc.dram_tensor("wo", [DH, D], F16, kind="ExternalInput")
    # konst [128, 256]: cols 0:128 = triu ones (f16), cols 128:256 = identity
    konst = nc.dram_tensor("konst", [128, 256], F16, kind="ExternalInput")
    outT = nc.dram_tensor("outT", [D, L], F16, kind="ExternalOutput")

    EXP = mybir.ActivationFunctionType.Exp

    with tile.TileContext(nc) as tc:
        from contextlib import ExitStack

        with ExitStack() as st:
            constp = st.enter_context(tc.tile_pool(name="const", bufs=1))
            ksb = constp.tile([128, 256], F16)
            nc.sync.dma_start(ksb[:], konst[:])
            ident_sb = ksb[:, 128:256]      # identity, f16
            ones64 = constp.tile([128, 64], F16)
            nc.vector.memset(ones64[:], 1.0)
            tri_bf = constp.tile([128, 128], BF16)
            nc.vector.tensor_copy(tri_bf[:], ksb[:, 0:128])

            actp = st.enter_context(tc.tile_pool(name="acts", bufs=1))
            qhT = actp.tile([128, NHP, L], F16)
            khT = actp.tile([128, NHP, L], F16)
            # vh: per 128-token chunk, 8 heads x (64 v-dims + ones col), bf16
            vh = actp.tile([128, TC, HPC * (HD + 1)], BF16)
            # whole tile starts at 1.0; projection copies overwrite the data
            # columns, leaving the 65th (denominator) column at 1.0
            nc.vector.memset(vh[:], 1.0)
            oT = actp.tile([128, NHP, L], F16)       # normalized per-head out
            # merged head-B staging + denominator rows, [65, qb%2, jl, 512]:
            # rows 0:64 of jl=2hp+1 hold head-B unnormalized o; row 64 holds
            # the denominator row for every block-head jl = hp*2 + h
            obden = actp.tile([65, 2, 8, QB], F16)

            # ---- persistent weights + per-section x tiles ----
            wqp = st.enter_context(tc.tile_pool(name="wqp", bufs=1))
            wkp = st.enter_context(tc.tile_pool(name="wkp", bufs=1))
            wvp = st.enter_context(tc.tile_pool(name="wvp", bufs=1))
            xqp = st.enter_context(tc.tile_pool(name="xqp", bufs=1))
            xkp = st.enter_context(tc.tile_pool(name="xkp", bufs=1))
            xvp = st.enter_context(tc.tile_pool(name="xvp", bufs=1))
            wq_sb = wqp.tile([128, KC, DH], F16)
            wk_sb = wkp.tile([128, KC, DH], F16)
            wv_sb = wvp.tile([128, KC, DH], F16)
            for w_sb, wdram in ((wq_sb, wq), (wk_sb, wk), (wv_sb, wv)):
                nc.sync.dma_start(
                    w_sb[:], wdram[:].rearrange("(kc p) m -> p kc m", p=128)
                )

            ppool = st.enter_context(tc.tile_pool(name="pp", bufs=4))
            projp = st.enter_context(
                tc.tile_pool(name="projp", bufs=2, space="PSUM")
            )
            spool = st.enter_context(
                tc.tile_pool(name="sps", bufs=2, space="PSUM")
            )
            opool = st.enter_context(
                tc.tile_pool(name="ops", bufs=2, space="PSUM")
            )
            rrpool = st.enter_context(tc.tile_pool(name="rrp", bufs=4))
            drpool = st.enter_context(tc.tile_pool(name="drp", bufs=2))
            wop = st.enter_context(tc.tile_pool(name="wop", bufs=1))
            osbp = st.enter_context(tc.tile_pool(name="osbp", bufs=4))
            wo_sb = wop.tile([128, NHP, D], F16)
            nc.sync.dma_start(
                wo_sb[:], wo[:].rearrange("(kc p) m -> p kc m", p=128)
            )

            def emit_outproj(qb):
                # output projection for query columns qb*QB:(qb+1)*QB
                for m in range(D // 128):
                    pt = projp.tile([128, QB], F32, tag="ps", name="pt")
                    for kc2 in range(NHP):
                        nc.tensor.matmul(
                            pt[:],
                            wo_sb[:, kc2, m * 128 : (m + 1) * 128],
                            oT[:, kc2, qb * QB : (qb + 1) * QB],
                            start=(kc2 == 0),
                            stop=(kc2 == NHP - 1),
                        )
                    osb = osbp.tile([128, QB], F16, tag="ot", name="osb")
                    if m % 2 == 0:
                        nc.vector.tensor_copy(osb[:], pt[:])
                    else:
                        nc.scalar.copy(osb[:], pt[:])
                    nc.sync.dma_start(
                        outT[m * 128 : (m + 1) * 128, qb * QB : (qb + 1) * QB],
                        osb[:],
                    )

            def emit_x_dma(n):
                tiles = {}
                for key, xp, xdram in (
                    ("q", xqp, xq), ("k", xkp, xk), ("v", xvp, xv)
                ):
                    x_sb = xp.tile([128, KC, QB], F16, tag="x", name=f"x{key}")
                    # issue from the ACT hwdge queue: overlaps the weight DMAs
                    # on the sync queue (parallel DMA streams at startup)
                    nc.scalar.dma_start(
                        x_sb[:],
                        xdram[:, n * QB : (n + 1) * QB].rearrange(
                            "(kc p) t -> p kc t", p=128
                        ),
                    )
                    tiles[key] = x_sb
                return tiles

            def emit_proj(n, xt):
                # q/k projections: head dims on partitions
                for w_sb, dst, key in (
                    (wq_sb, qhT, "q"), (wk_sb, khT, "k")
                ):
                    for m in range(NHP):
                        ps = projp.tile([128, QB], F32, tag="ps", name="ps")
                        for kc in range(KC):
                            nc.tensor.matmul(
                                ps[:],
                                w_sb[:, kc, m * 128 : (m + 1) * 128],
                                xt[key][:, kc, :],
                                start=(kc == 0),
                                stop=(kc == KC - 1),
                            )
                        nc.vector.tensor_copy(
                            dst[:, m, n * QB : (n + 1) * QB], ps[:]
                        )
                # v projection: tokens on partitions
                for tt in range(KPQ):
                    ps = projp.tile([128, QB], F32, tag="ps", name="ps")
                    for kc in range(KC):
                        nc.tensor.matmul(
                            ps[:, 0:DH],
                            xt["v"][:, kc, tt * 128 : (tt + 1) * 128],
                            wv_sb[:, kc, :],
                            start=(kc == 0),
                            stop=(kc == KC - 1),
                        )
                    tci = n * KPQ + tt
                    vdst = vh[:, tci, :].rearrange("p (h c) -> p h c", c=HD + 1)
                    nc.vector.tensor_copy(
                        vdst[:, :, 0:HD],
                        ps[:, 0:DH].rearrange("p (h d) -> p h d", d=HD),
                    )

            # ---- SDPA section for q-block qb: blocks (hp, qb), lag-2 ----
            def emit_sdpa_section(qb):
                kcnt = (qb + 1) * KPQ
                q0 = qb * QB
                steps = [(hp, kc) for hp in range(NHP) for kc in range(kcnt)]
                state = {}

                def emit_s(i):
                    hp, kc = steps[i]
                    if hp not in state:
                        state[hp] = {"p": {}, "o": None}
                    stt = state[hp]
                    dj = kc - qb * KPQ
                    c0 = 128 * dj if dj > 0 else 0
                    spair = spool.tile([128, 2, QB], F32, tag="s", name="spair")
                    nc.tensor.matmul(
                        spair[:, 0, c0:QB],
                        khT[0:64, hp, kc * 128 : (kc + 1) * 128],
                        qhT[0:64, hp, q0 + c0 : q0 + QB],
                        start=True,
                        stop=True,
                    )
                    nc.tensor.matmul(
                        spair[:, 1, c0:QB],
                        khT[64:128, hp, kc * 128 : (kc + 1) * 128],
                        qhT[64:128, hp, q0 + c0 : q0 + QB],
                        start=True,
                        stop=True,
                    )
                    ppair = ppool.tile([128, 2, QB], BF16, tag="p", name="ppair")
                    nc.scalar.activation(
                        ppair[:, :, c0:QB], spair[:, :, c0:QB], EXP
                    )
                    if dj >= 0:
                        for h in range(2):
                            nc.vector.tensor_mul(
                                ppair[:, h, c0 : c0 + 128],
                                ppair[:, h, c0 : c0 + 128],
                                tri_bf[:],
                            )
                    stt["p"][kc] = ppair

                def emit_pv(i):
                    hp, kc = steps[i]
                    stt = state[hp]
                    dj = kc - qb * KPQ
                    c0 = 128 * dj if dj > 0 else 0
                    if kc == 0:
                        stt["o"] = (
                            opool.tile([65, QB], F32, tag="o", name="o_a"),
                            opool.tile([65, QB], F32, tag="o", name="o_b"),
                        )
                    ppair = stt["p"].pop(kc)
                    for h in range(2):
                        nc.tensor.matmul(
                            stt["o"][h][:, c0:QB],
                            vh[:, kc, (2 * hp + h) * (HD + 1) : (2 * hp + h + 1) * (HD + 1)],
                            ppair[:, h, c0:QB],
                            start=(kc == 0),
                            stop=(kc == kcnt - 1),
                        )
                    if kc == kcnt - 1:
                        # unnormalized o -> SBUF; denominator rows -> obden
                        stt = state.pop(hp)
                        oA, oB = stt["o"]
                        nc.vector.tensor_copy(
                            oT[0:64, hp, q0 : q0 + QB], oA[0:64, :]
                        )
                        nc.vector.tensor_copy(
                            obden[64:65, qb % 2, hp * 2, :], oA[64:65, :]
                        )
                        nc.vector.tensor_copy(
                            obden[0:65, qb % 2, hp * 2 + 1, :], oB[0:65, :]
                        )

                LAG = 2
                for i in range(len(steps) + LAG):
                    if i < len(steps):
                        emit_s(i)
                    if i - LAG >= 0:
                        emit_pv(i - LAG)

            # ---- deferred normalization for section qb ----
            def emit_norm(qb):
                q0 = qb * QB
                den_t = projp.tile([128, QB], F32, tag="ps", name="den_t")
                for jl in range(8):              # jl = hp*2 + head
                    for c in range(4):
                        nc.tensor.matmul(
                            den_t[:, jl * 4 + c : jl * 4 + c + 1],
                            obden[64:65, qb % 2, jl, c * 128 : (c + 1) * 128],
                            ones64[64:65, 0:1],
                            start=(jl == 0 and c == 0),
                            stop=(jl == 7 and c == 3),
                            skip_group_check=True,
                        )
                den_rt = drpool.tile([128, 32], F32, tag="dr", name="den_rt")
                nc.vector.reciprocal(den_rt[:], den_t[:, 0:32])
                for jl in range(8):
                    hp, h = jl // 2, jl % 2
                    bc = projp.tile([128, QB], F32, tag="ps", name="bc")
                    for c in range(4):
                        rrep = rrpool.tile([128, 64], F16, tag="rr", name="rrep")
                        nc.vector.tensor_scalar_mul(
                            rrep[:],
                            ones64[:],
                            den_rt[:, jl * 4 + c : jl * 4 + c + 1],
                        )
                        nc.tensor.matmul(
                            bc[0:64, c * 128 : (c + 1) * 128],
                            rrep[:],
                            ident_sb[:],
                            start=(c == 0),
                            stop=(c == 3),
                            skip_group_check=True,
                        )
                    if h == 0:
                        dst = oT[0:64, hp, q0 : q0 + QB]
                        nc.vector.tensor_mul(dst, dst, bc[0:64, :])
                    else:
                        dst = obden[0:64, qb % 2, hp * 2 + 1, :]
                        nc.vector.tensor_mul(dst, dst, bc[0:64, :])
                        nc.sync.dma_start(
                            oT[64:128, hp, q0 : q0 + QB], dst
                        )

            # ---- main interleaved pipeline ----
            xt = emit_x_dma(0)
            for n in range(NT):
                emit_proj(n, xt)
                if n + 1 < NT:
                    xt = emit_x_dma(n + 1)
                if n >= 1:
                    emit_norm(n - 1)
                emit_sdpa_section(n)
                if n >= 1:
                    emit_outproj(n - 1)
            emit_norm(NT - 1)
            emit_outproj(NT - 1)

    return nc


def build():
    nc = bacc.Bacc("TRN2", target_bir_lowering=False, debug=False)
    _emit(nc)
    nc.compile()
    return nc


_NC_CACHE = {}


def _get_nc():
    if "nc" not in _NC_CACHE:
        _NC_CACHE["nc"] = build()
    return _NC_CACHE["nc"]


def make_in_maps(q, k, v, Wq, Wk, Wv, Wo):
    konst_m = np.zeros((128, 256), dtype=np.float16)
    konst_m[:, 0:128] = np.triu(np.ones((128, 128), dtype=np.float16))
    konst_m[:, 128:256] = np.eye(128, dtype=np.float16)
    qT = [np.ascontiguousarray(q[b].T).astype(np.float16) for b in range(B)]
    kT = [np.ascontiguousarray(k[b].T).astype(np.float16) for b in range(B)]
    vT = [np.ascontiguousarray(v[b].T).astype(np.float16) for b in range(B)]
    wq_s, wk_s, wv_s, wo_s = [], [], [], []
    for tp in range(TP):
        rows = slice(tp * DH, (tp + 1) * DH)
        wq_s.append(np.ascontiguousarray(Wq[rows].T * SCALE).astype(np.float16))
        wk_s.append(np.ascontiguousarray(Wk[rows].T).astype(np.float16))
        wv_s.append(np.ascontiguousarray(Wv[rows].T).astype(np.float16))
        wo_s.append(np.ascontiguousarray(Wo[:, rows].T).astype(np.float16))
    in_maps = []
    for c in range(N_CORES):
        b, tp = c // TP, c % TP
        in_maps.append(
            {
                "xq": qT[b],
                "xk": kT[b],
                "xv": vT[b],
                "wq": wq_s[tp],
                "wk": wk_s[tp],
                "wv": wv_s[tp],
                "wo": wo_s[tp],
                "konst": konst_m,
            }
        )
    return in_maps


def kernel(q, k, v, Wq, Wk, Wv, Wo, mask=None, trace=False):
    q = np.asarray(q, dtype=np.float32)
    k = np.asarray(k, dtype=np.float32)
    v = np.asarray(v, dtype=np.float32)
    nc = _get_nc()
    in_maps = make_in_maps(
        q, k, v,
        np.asarray(Wq, np.float32), np.asarray(Wk, np.float32),
        np.asarray(Wv, np.float32), np.asarray(Wo, np.float32),
    )
    res = run_bass_kernel_spmd(
        nc, in_maps, core_ids=list(range(N_CORES)), trace=trace
    )
    out = np.zeros((B, L, D), dtype=np.float32)
    for c in range(N_CORES):
        out[c // TP] += res.results[c]["outT"].T.astype(np.float32)
    if trace:
        return out, res
    return out


# revision 13
# speedup vs baseline: 1.0391x; 1.0225x over previous
"""Causal multi-head attention (B=2, L=2048, D=2048, H=32) on 8 trn2 NeuronCores.

Sharding: data-parallel over batch (2 groups of 4 cores) x tensor-parallel over
heads (8 heads per core). Host pre-transposes x and pre-shards/scales weights;
host sums the 4 tensor-parallel partials per batch (fp32).

v3 design:
  - fp16 operands (bf16 for p/vh so exp of unmasked garbage can't overflow
    16-bit range); fp32 PSUM accumulation; fp16 output partials.
  - Fully interleaved token-tile pipeline: for each 512-token tile n, emit the
    q/k/v projections for tile n and then SDPA for q-block n (causality means
    its whole k-range is already projected).  ACT's exp stream overlaps the
    next tile's projection matmuls, so the PE never waits out the softmax.
  - x-tile DMAs emitted one section early (prefetch during SDPA).
  - S matmuls run as head PAIRS on row-tiles (0,0)/(64,0) (K=64 concurrent);
    exp batched over [128,2,512] PSUM double-tiles; PV restricted to the
    causally valid columns; lag-2 software pipeline S->exp->PV.
  - softmax denominators: per-block rows collected at partition 64, PE-gathered
    onto partitions, ONE batched reciprocal per q-section [128,32], PE
    broadcast back; normalization deferred one section (overlaps projections).
  - ACT is exp-only during the pipeline; projection PSUM copies + triangle
    masking on DVE; outproj copies split DVE/ACT.
"""

import sys

sys.path.insert(0, "/opt/trn_rl_repo")

import numpy as np

import concourse.bass as bass
import concourse.tile as tile
from concourse import bacc, mybir
from concourse.bass_utils import run_bass_kernel_spmd


def _ensure_ntff_hook():
    """The agent image's antenv package lacks axon_hooks, which makes
    run_bass_kernel_spmd(trace=True) crash on import. Provide the module and
    register the ctypes-based NTFF profiling hook (degrades silently)."""
    try:
        import types

        import antenv

        if "antenv.axon_hooks" not in sys.modules:
            m = types.ModuleType("antenv.axon_hooks")
            state = {"hook": None}
            m.set_axon_ntff_profile_hook = lambda h: state.__setitem__("hook", h)
            m.get_axon_ntff_profile_hook = lambda: state["hook"]
            sys.modules["antenv.axon_hooks"] = m
            antenv.axon_hooks = m
        from antenv.axon_hooks import (
            get_axon_ntff_profile_hook,
            set_axon_ntff_profile_hook,
        )

        if get_axon_ntff_profile_hook() is None:
            from trn_agent_boot.trn_boot import _ntff_profile_via_ctypes

            set_axon_ntff_profile_hook(
                _ntff_profile_via_ctypes("/opt/axon/libaxon_pjrt.so")
            )
    except Exception:
        pass


_ensure_ntff_hook()

F32 = mybir.dt.float32
F16 = mybir.dt.float16
BF16 = mybir.dt.bfloat16

B, L, D, H = 2, 2048, 2048, 32
HD = 64          # head dim
N_CORES = 8
TP = 4           # tensor-parallel width (heads split 4 ways)
HPC = H // TP    # heads per core = 8
NHP = HPC // 2   # head pairs per core = 4
DH = HPC * HD    # per-core projected width = 512
SCALE = float(HD) ** -0.5

QB = 512         # query-block width for SDPA
KC = D // 128    # contraction chunks for the projections = 16
NT = L // QB     # 512-token tiles = 4
TC = L // 128    # 128-token chunks = 16
KPQ = QB // 128  # k-chunks per q-block = 4


def _emit(nc):
    # x: [128, section, kc, 512] so each section tile is one contiguous
    # 16KB-per-partition DMA; w: [128, kc, m] partition-major, same reason
    xq = nc.dram_tensor("xq", [128, NT, KC, QB], F16, kind="ExternalInput")
    xk = nc.dram_tensor("xk", [128, NT, KC, QB], F16, kind="ExternalInput")
    xv = nc.dram_tensor("xv", [128, NT, KC, QB], F16, kind="ExternalInput")
    wq = nc.dram_tensor("wq", [128, KC, DH], F16, kind="ExternalInput")
    wk = nc.dram_tensor("wk", [128, KC, DH], F16, kind="ExternalInput")
    wv = nc.dram_tensor("wv", [128, KC, DH], F16, kind="ExternalInput")
    wo = nc.dram_tensor("wo", [128, NHP, D], F16, kind="ExternalInput")
    # konst [128, 256]: cols 0:128 = triu ones (f16), cols 128:256 = identity
    konst = nc.dram_tensor("konst", [128, 256], F16, kind="ExternalInput")
    outT = nc.dram_tensor("outT", [D, L], F16, kind="ExternalOutput")

    EXP = mybir.ActivationFunctionType.Exp

    with tile.TileContext(nc) as tc:
        from contextlib import ExitStack

        with ExitStack() as st:
            constp = st.enter_context(tc.tile_pool(name="const", bufs=1))
            ksb = constp.tile([128, 256], F16)
            nc.sync.dma_start(ksb[:], konst[:])
            ident_sb = ksb[:, 128:256]      # identity, f16
            ones64 = constp.tile([128, 64], F16)
            nc.vector.memset(ones64[:], 1.0)
            tri_bf = constp.tile([128, 128], BF16)
            nc.vector.tensor_copy(tri_bf[:], ksb[:, 0:128])

            actp = st.enter_context(tc.tile_pool(name="acts", bufs=1))
            qhT = actp.tile([128, NHP, L], F16)
            khT = actp.tile([128, NHP, L], F16)
            # vh: per 128-token chunk, 8 heads x (64 v-dims + ones col), bf16
            vh = actp.tile([128, TC, HPC * (HD + 1)], BF16)
            # whole tile starts at 1.0; projection copies overwrite the data
            # columns, leaving the 65th (denominator) column at 1.0
            nc.vector.memset(vh[:], 1.0)
            oT = actp.tile([128, NHP, L], F16)       # normalized per-head out
            # merged head-B staging + denominator rows, [65, qb%2, jl, 512]:
            # rows 0:64 of jl=2hp+1 hold head-B unnormalized o; row 64 holds
            # the denominator row for every block-head jl = hp*2 + h
            obden = actp.tile([65, 2, 8, QB], F16)

            # ---- persistent weights + per-section x tiles ----
            wqp = st.enter_context(tc.tile_pool(name="wqp", bufs=1))
            wkp = st.enter_context(tc.tile_pool(name="wkp", bufs=1))
            wvp = st.enter_context(tc.tile_pool(name="wvp", bufs=1))
            xqp = st.enter_context(tc.tile_pool(name="xqp", bufs=1))
            xkp = st.enter_context(tc.tile_pool(name="xkp", bufs=1))
            xvp = st.enter_context(tc.tile_pool(name="xvp", bufs=1))
            wq_sb = wqp.tile([128, KC, DH], F16)
            wk_sb = wkp.tile([128, KC, DH], F16)
            wv_sb = wvp.tile([128, KC, DH], F16)
            for w_sb, wdram in ((wq_sb, wq), (wk_sb, wk), (wv_sb, wv)):
                nc.sync.dma_start(w_sb[:], wdram[:])

            ppool = st.enter_context(tc.tile_pool(name="pp", bufs=4))
            projp = st.enter_context(
                tc.tile_pool(name="projp", bufs=2, space="PSUM")
            )
            spool = st.enter_context(
                tc.tile_pool(name="sps", bufs=2, space="PSUM")
            )
            opool = st.enter_context(
                tc.tile_pool(name="ops", bufs=2, space="PSUM")
            )
            rrpool = st.enter_context(tc.tile_pool(name="rrp", bufs=4))
            drpool = st.enter_context(tc.tile_pool(name="drp", bufs=2))
            wop = st.enter_context(tc.tile_pool(name="wop", bufs=1))
            osbp = st.enter_context(tc.tile_pool(name="osbp", bufs=4))
            wo_sb = wop.tile([128, NHP, D], F16)
            nc.sync.dma_start(wo_sb[:], wo[:])

            def emit_outproj(qb):
                # output projection for query columns qb*QB:(qb+1)*QB
                for m in range(D // 128):
                    pt = projp.tile([128, QB], F32, tag="ps", name="pt")
                    for kc2 in range(NHP):
                        nc.tensor.matmul(
                            pt[:],
                            wo_sb[:, kc2, m * 128 : (m + 1) * 128],
                            oT[:, kc2, qb * QB : (qb + 1) * QB],
                            start=(kc2 == 0),
                            stop=(kc2 == NHP - 1),
                        )
                    osb = osbp.tile([128, QB], F16, tag="ot", name="osb")
                    if m % 2 == 0:
                        nc.vector.tensor_copy(osb[:], pt[:])
                    else:
                        nc.scalar.copy(osb[:], pt[:])
                    nc.sync.dma_start(
                        outT[m * 128 : (m + 1) * 128, qb * QB : (qb + 1) * QB],
                        osb[:],
                    )

            def emit_x_dma(n):
                tiles = {}
                eng = nc.scalar if n == 0 else nc.sync
                for key, xp, xdram in (
                    ("q", xqp, xq), ("k", xkp, xk), ("v", xvp, xv)
                ):
                    x_sb = xp.tile([128, KC, QB], F16, tag="x", name=f"x{key}")
                    eng.dma_start(x_sb[:], xdram[:, n, :, :])
                    tiles[key] = x_sb
                return tiles

            def emit_proj(n, xt):
                # q/k projections: head dims on partitions
                for w_sb, dst, key in (
                    (wq_sb, qhT, "q"), (wk_sb, khT, "k")
                ):
                    for m in range(NHP):
                        ps = projp.tile([128, QB], F32, tag="ps", name="ps")
                        for kc in range(KC):
                            nc.tensor.matmul(
                                ps[:],
                                w_sb[:, kc, m * 128 : (m + 1) * 128],
                                xt[key][:, kc, :],
                                start=(kc == 0),
                                stop=(kc == KC - 1),
                            )
                        nc.vector.tensor_copy(
                            dst[:, m, n * QB : (n + 1) * QB], ps[:]
                        )
                # v projection: tokens on partitions
                for tt in range(KPQ):
                    ps = projp.tile([128, QB], F32, tag="ps", name="ps")
                    for kc in range(KC):
                        nc.tensor.matmul(
                            ps[:, 0:DH],
                            xt["v"][:, kc, tt * 128 : (tt + 1) * 128],
                            wv_sb[:, kc, :],
                            start=(kc == 0),
                            stop=(kc == KC - 1),
                        )
                    tci = n * KPQ + tt
                    vdst = vh[:, tci, :].rearrange("p (h c) -> p h c", c=HD + 1)
                    nc.vector.tensor_copy(
                        vdst[:, :, 0:HD],
                        ps[:, 0:DH].rearrange("p (h d) -> p h d", d=HD),
                    )

            # ---- SDPA section for q-block qb: blocks (hp, qb), lag-2 ----
            def emit_sdpa_section(qb):
                kcnt = (qb + 1) * KPQ
                q0 = qb * QB
                steps = [(hp, kc) for hp in range(NHP) for kc in range(kcnt)]
                state = {}

                def emit_s(i):
                    hp, kc = steps[i]
                    if hp not in state:
                        state[hp] = {"p": {}, "o": None}
                    stt = state[hp]
                    dj = kc - qb * KPQ
                    c0 = 128 * dj if dj > 0 else 0
                    spair = spool.tile([128, 2, QB], F32, tag="s", name="spair")
                    nc.tensor.matmul(
                        spair[:, 0, c0:QB],
                        khT[0:64, hp, kc * 128 : (kc + 1) * 128],
                        qhT[0:64, hp, q0 + c0 : q0 + QB],
                        start=True,
                        stop=True,
                    )
                    nc.tensor.matmul(
                        spair[:, 1, c0:QB],
                        khT[64:128, hp, kc * 128 : (kc + 1) * 128],
                        qhT[64:128, hp, q0 + c0 : q0 + QB],
                        start=True,
                        stop=True,
                    )
                    ppair = ppool.tile([128, 2, QB], BF16, tag="p", name="ppair")
                    nc.scalar.activation(
                        ppair[:, :, c0:QB], spair[:, :, c0:QB], EXP
                    )
                    if dj >= 0:
                        for h in range(2):
                            nc.vector.tensor_mul(
                                ppair[:, h, c0 : c0 + 128],
                                ppair[:, h, c0 : c0 + 128],
                                tri_bf[:],
                            )
                    stt["p"][kc] = ppair

                def emit_pv(i):
                    hp, kc = steps[i]
                    stt = state[hp]
                    dj = kc - qb * KPQ
                    c0 = 128 * dj if dj > 0 else 0
                    if kc == 0:
                        stt["o"] = (
                            opool.tile([65, QB], F32, tag="o", name="o_a"),
                            opool.tile([65, QB], F32, tag="o", name="o_b"),
                        )
                    ppair = stt["p"].pop(kc)
                    for h in range(2):
                        nc.tensor.matmul(
                            stt["o"][h][:, c0:QB],
                            vh[:, kc, (2 * hp + h) * (HD + 1) : (2 * hp + h + 1) * (HD + 1)],
                            ppair[:, h, c0:QB],
                            start=(kc == 0),
                            stop=(kc == kcnt - 1),
                        )
                    if kc == kcnt - 1:
                        # unnormalized o -> SBUF; denominator rows -> obden
                        stt = state.pop(hp)
                        oA, oB = stt["o"]
                        nc.vector.tensor_copy(
                            oT[0:64, hp, q0 : q0 + QB], oA[0:64, :]
                        )
                        nc.vector.tensor_copy(
                            obden[64:65, qb % 2, hp * 2, :], oA[64:65, :]
                        )
                        nc.vector.tensor_copy(
                            obden[0:65, qb % 2, hp * 2 + 1, :], oB[0:65, :]
                        )

                LAG = 2
                for i in range(len(steps) + LAG):
                    if i < len(steps):
                        emit_s(i)
                    if i - LAG >= 0:
                        emit_pv(i - LAG)

            # ---- deferred normalization for section qb ----
            def emit_norm(qb):
                q0 = qb * QB
                den_t = projp.tile([128, QB], F32, tag="ps", name="den_t")
                for jl in range(8):              # jl = hp*2 + head
                    for c in range(4):
                        nc.tensor.matmul(
                            den_t[:, jl * 4 + c : jl * 4 + c + 1],
                            obden[64:65, qb % 2, jl, c * 128 : (c + 1) * 128],
                            ones64[64:65, 0:1],
                            start=(jl == 0 and c == 0),
                            stop=(jl == 7 and c == 3),
                            skip_group_check=True,
                        )
                den_rt = drpool.tile([128, 32], F32, tag="dr", name="den_rt")
                nc.vector.reciprocal(den_rt[:], den_t[:, 0:32])
                for jl in range(8):
                    hp, h = jl // 2, jl % 2
                    bc = projp.tile([128, QB], F32, tag="ps", name="bc")
                    for c in range(4):
                        rrep = rrpool.tile([128, 64], F16, tag="rr", name="rrep")
                        nc.vector.tensor_scalar_mul(
                            rrep[:],
                            ones64[:],
                            den_rt[:, jl * 4 + c : jl * 4 + c + 1],
                        )
                        nc.tensor.matmul(
                            bc[0:64, c * 128 : (c + 1) * 128],
                            rrep[:],
                            ident_sb[:],
                            start=(c == 0),
                            stop=(c == 3),
                            skip_group_check=True,
                        )
                    if h == 0:
                        dst = oT[0:64, hp, q0 : q0 + QB]
                        nc.vector.tensor_mul(dst, dst, bc[0:64, :])
                    else:
                        dst = obden[0:64, qb % 2, hp * 2 + 1, :]
                        nc.vector.tensor_mul(dst, dst, bc[0:64, :])
                        nc.sync.dma_start(
                            oT[64:128, hp, q0 : q0 + QB], dst
                        )

            # ---- main interleaved pipeline ----
            xt = emit_x_dma(0)
            for n in range(NT):
                emit_proj(n, xt)
                if n + 1 < NT:
                    xt = emit_x_dma(n + 1)
                if n >= 1:
                    emit_norm(n - 1)
                emit_sdpa_section(n)
                if n >= 1:
                    emit_outproj(n - 1)
            emit_norm(NT - 1)
            emit_outproj(NT - 1)

    return nc


def build():
    nc = bacc.Bacc("TRN2", target_bir_lowering=False, debug=False)
    _emit(nc)
    nc.compile()
    return nc


_NC_CACHE = {}


def _get_nc():
    if "nc" not in _NC_CACHE:
        _NC_CACHE["nc"] = build()
    return _NC_CACHE["nc"]


def make_in_maps(q, k, v, Wq, Wk, Wv, Wo):
    konst_m = np.zeros((128, 256), dtype=np.float16)
    konst_m[:, 0:128] = np.triu(np.ones((128, 128), dtype=np.float16))
    konst_m[:, 128:256] = np.eye(128, dtype=np.float16)
    def xlay(xb):
        # x[b] [L, D] -> xT [D, L] -> [kc, 128, NT, QB] -> [128, NT, kc, QB]
        xT = xb.T.reshape(KC, 128, NT, QB)
        return np.ascontiguousarray(
            xT.transpose(1, 2, 0, 3)
        ).astype(np.float16)

    def wlay(wt):
        # wt [D, DH_like] -> [kc, 128, M] -> [128, kc, M]
        m = wt.shape[1]
        return np.ascontiguousarray(
            wt.reshape(KC, 128, m).transpose(1, 0, 2)
        ).astype(np.float16)

    qT = [xlay(np.asarray(q[b])) for b in range(B)]
    kT = [xlay(np.asarray(k[b])) for b in range(B)]
    vT = [xlay(np.asarray(v[b])) for b in range(B)]
    wq_s, wk_s, wv_s, wo_s = [], [], [], []
    for tp in range(TP):
        rows = slice(tp * DH, (tp + 1) * DH)
        wq_s.append(wlay(Wq[rows].T * SCALE))
        wk_s.append(wlay(Wk[rows].T))
        wv_s.append(wlay(Wv[rows].T))
        # wo [DH, D]: DH = 4 chunks of 128 -> [128, NHP, D]
        wo_s.append(np.ascontiguousarray(
            Wo[:, rows].T.reshape(NHP, 128, D).transpose(1, 0, 2)
        ).astype(np.float16))
    in_maps = []
    for c in range(N_CORES):
        b, tp = c // TP, c % TP
        in_maps.append(
            {
                "xq": qT[b],
                "xk": kT[b],
                "xv": vT[b],
                "wq": wq_s[tp],
                "wk": wk_s[tp],
                "wv": wv_s[tp],
                "wo": wo_s[tp],
                "konst": konst_m,
            }
        )
    return in_maps


def kernel(q, k, v, Wq, Wk, Wv, Wo, mask=None, trace=False):
    q = np.asarray(q, dtype=np.float32)
    k = np.asarray(k, dtype=np.float32)
    v = np.asarray(v, dtype=np.float32)
    nc = _get_nc()
    in_maps = make_in_maps(
        q, k, v,
        np.asarray(Wq, np.float32), np.asarray(Wk, np.float32),
        np.asarray(Wv, np.float32), np.asarray(Wo, np.float32),
    )
    res = run_bass_kernel_spmd(
        nc, in_maps, core_ids=list(range(N_CORES)), trace=trace
    )
    out = np.zeros((B, L, D), dtype=np.float32)
    for c in range(N_CORES):
        out[c // TP] += res.results[c]["outT"].T.astype(np.float32)
    if trace:
        return out, res
    return out


# revision 14
# speedup vs baseline: 1.0656x; 1.0255x over previous
"""Causal multi-head attention (B=2, L=2048, D=2048, H=32) on 8 trn2 NeuronCores.

Sharding: data-parallel over batch (2 groups of 4 cores) x tensor-parallel over
heads (8 heads per core). Host pre-transposes x and pre-shards/scales weights;
host sums the 4 tensor-parallel partials per batch (fp32).

v3 design:
  - fp16 operands (bf16 for p/vh so exp of unmasked garbage can't overflow
    16-bit range); fp32 PSUM accumulation; fp16 output partials.
  - Fully interleaved token-tile pipeline: for each 512-token tile n, emit the
    q/k/v projections for tile n and then SDPA for q-block n (causality means
    its whole k-range is already projected).  ACT's exp stream overlaps the
    next tile's projection matmuls, so the PE never waits out the softmax.
  - x-tile DMAs emitted one section early (prefetch during SDPA).
  - S matmuls run as head PAIRS on row-tiles (0,0)/(64,0) (K=64 concurrent);
    exp batched over [128,2,512] PSUM double-tiles; PV restricted to the
    causally valid columns; lag-2 software pipeline S->exp->PV.
  - softmax denominators: per-block rows collected at partition 64, PE-gathered
    onto partitions, ONE batched reciprocal per q-section [128,32], PE
    broadcast back; normalization deferred one section (overlaps projections).
  - ACT is exp-only during the pipeline; projection PSUM copies + triangle
    masking on DVE; outproj copies split DVE/ACT.
"""

import sys

sys.path.insert(0, "/opt/trn_rl_repo")

import numpy as np

import concourse.bass as bass
import concourse.tile as tile
from concourse import bacc, mybir
from concourse.bass_utils import run_bass_kernel_spmd


def _ensure_ntff_hook():
    """The agent image's antenv package lacks axon_hooks, which makes
    run_bass_kernel_spmd(trace=True) crash on import. Provide the module and
    register the ctypes-based NTFF profiling hook (degrades silently)."""
    try:
        import types

        import antenv

        if "antenv.axon_hooks" not in sys.modules:
            m = types.ModuleType("antenv.axon_hooks")
            state = {"hook": None}
            m.set_axon_ntff_profile_hook = lambda h: state.__setitem__("hook", h)
            m.get_axon_ntff_profile_hook = lambda: state["hook"]
            sys.modules["antenv.axon_hooks"] = m
            antenv.axon_hooks = m
        from antenv.axon_hooks import (
            get_axon_ntff_profile_hook,
            set_axon_ntff_profile_hook,
        )

        if get_axon_ntff_profile_hook() is None:
            from trn_agent_boot.trn_boot import _ntff_profile_via_ctypes

            set_axon_ntff_profile_hook(
                _ntff_profile_via_ctypes("/opt/axon/libaxon_pjrt.so")
            )
    except Exception:
        pass


_ensure_ntff_hook()

F32 = mybir.dt.float32
F16 = mybir.dt.float16
BF16 = mybir.dt.bfloat16

B, L, D, H = 2, 2048, 2048, 32
HD = 64          # head dim
N_CORES = 8
TP = 4           # tensor-parallel width (heads split 4 ways)
HPC = H // TP    # heads per core = 8
NHP = HPC // 2   # head pairs per core = 4
DH = HPC * HD    # per-core projected width = 512
SCALE = float(HD) ** -0.5

QB = 512         # query-block width for SDPA
KC = D // 128    # contraction chunks for the projections = 16
NT = L // QB     # 512-token tiles = 4
TC = L // 128    # 128-token chunks = 16
KPQ = QB // 128  # k-chunks per q-block = 4


def _emit(nc):
    # x: [128, section, kc, 512] so each section tile is one contiguous
    # 16KB-per-partition DMA; w: [128, kc, m] partition-major, same reason
    xq = nc.dram_tensor("xq", [128, NT, KC, QB], F16, kind="ExternalInput")
    xk = nc.dram_tensor("xk", [128, NT, KC, QB], F16, kind="ExternalInput")
    xv = nc.dram_tensor("xv", [128, NT, KC, QB], F16, kind="ExternalInput")
    wq = nc.dram_tensor("wq", [128, KC, DH], F16, kind="ExternalInput")
    wk = nc.dram_tensor("wk", [128, KC, DH], F16, kind="ExternalInput")
    wv = nc.dram_tensor("wv", [128, KC, DH], F16, kind="ExternalInput")
    wo = nc.dram_tensor("wo", [128, NHP, D], F16, kind="ExternalInput")
    # konst [128, 256]: cols 0:128 = triu ones (f16), cols 128:256 = identity
    konst = nc.dram_tensor("konst", [128, 256], F16, kind="ExternalInput")
    outT = nc.dram_tensor("outT", [D, L], F16, kind="ExternalOutput")

    EXP = mybir.ActivationFunctionType.Exp

    with tile.TileContext(nc) as tc:
        from contextlib import ExitStack

        with ExitStack() as st:
            constp = st.enter_context(tc.tile_pool(name="const", bufs=1))
            ksb = constp.tile([128, 256], F16)
            nc.sync.dma_start(ksb[:], konst[:])
            ident_sb = ksb[:, 128:256]      # identity, f16
            ones64 = constp.tile([128, 64], F16)
            nc.vector.memset(ones64[:], 1.0)
            tri_bf = constp.tile([128, 128], BF16)
            nc.vector.tensor_copy(tri_bf[:], ksb[:, 0:128])

            actp = st.enter_context(tc.tile_pool(name="acts", bufs=1))
            qhT = actp.tile([128, NHP, L], F16)
            khT = actp.tile([128, NHP, L], F16)
            # vh: per 128-token chunk, 8 heads x (64 v-dims + ones col), bf16
            vh = actp.tile([128, TC, HPC * (HD + 1)], BF16)
            # whole tile starts at 1.0; projection copies overwrite the data
            # columns, leaving the 65th (denominator) column at 1.0
            nc.vector.memset(vh[:], 1.0)
            oT = actp.tile([128, NHP, L], F16)       # normalized per-head out
            # merged head-B staging + denominator rows, [65, qb%2, jl, 512]:
            # rows 0:64 of jl=2hp+1 hold head-B unnormalized o; row 64 holds
            # the denominator row for every block-head jl = hp*2 + h
            obden = actp.tile([65, 2, 8, QB], F16)

            # ---- persistent weights + per-section x tiles ----
            wqp = st.enter_context(tc.tile_pool(name="wqp", bufs=1))
            wkp = st.enter_context(tc.tile_pool(name="wkp", bufs=1))
            wvp = st.enter_context(tc.tile_pool(name="wvp", bufs=1))
            xqp = st.enter_context(tc.tile_pool(name="xqp", bufs=1))
            xkp = st.enter_context(tc.tile_pool(name="xkp", bufs=1))
            xvp = st.enter_context(tc.tile_pool(name="xvp", bufs=1))
            wq_sb = wqp.tile([128, KC, DH], F16)
            wk_sb = wkp.tile([128, KC, DH], F16)
            wv_sb = wvp.tile([128, KC, DH], F16)
            for w_sb, wdram in ((wq_sb, wq), (wk_sb, wk), (wv_sb, wv)):
                nc.sync.dma_start(w_sb[:], wdram[:])

            ppool = st.enter_context(tc.tile_pool(name="pp", bufs=4))
            projp = st.enter_context(
                tc.tile_pool(name="projp", bufs=2, space="PSUM")
            )
            spool = st.enter_context(
                tc.tile_pool(name="sps", bufs=2, space="PSUM")
            )
            opool = st.enter_context(
                tc.tile_pool(name="ops", bufs=2, space="PSUM")
            )
            rrpool = st.enter_context(tc.tile_pool(name="rrp", bufs=4))
            drpool = st.enter_context(tc.tile_pool(name="drp", bufs=2))
            wop = st.enter_context(tc.tile_pool(name="wop", bufs=1))
            osbp = st.enter_context(tc.tile_pool(name="osbp", bufs=4))
            wo_sb = wop.tile([128, NHP, D], F16)
            nc.sync.dma_start(wo_sb[:], wo[:])

            def outproj_group(qb, m):
                # output projection for one 128-row chunk of query cols qb
                def clo():
                    pt = projp.tile([128, QB], F32, tag="ps", name="pt")
                    for kc2 in range(NHP):
                        nc.tensor.matmul(
                            pt[:],
                            wo_sb[:, kc2, m * 128 : (m + 1) * 128],
                            oT[:, kc2, qb * QB : (qb + 1) * QB],
                            start=(kc2 == 0),
                            stop=(kc2 == NHP - 1),
                        )
                    osb = osbp.tile([128, QB], F16, tag="ot", name="osb")
                    nc.vector.tensor_copy(osb[:], pt[:])
                    nc.sync.dma_start(
                        outT[m * 128 : (m + 1) * 128, qb * QB : (qb + 1) * QB],
                        osb[:],
                    )
                return clo

            def emit_x_dma(n):
                tiles = {}
                eng = nc.scalar if n == 0 else nc.sync
                for key, xp, xdram in (
                    ("q", xqp, xq), ("k", xkp, xk), ("v", xvp, xv)
                ):
                    x_sb = xp.tile([128, KC, QB], F16, tag="x", name=f"x{key}")
                    eng.dma_start(x_sb[:], xdram[:, n, :, :])
                    tiles[key] = x_sb
                return tiles

            def proj_group(n, xt, kind, m):
                # one m-group (or token-group for v) of a projection: 16
                # accumulating MMs into one psum tile + a DVE copy out
                def clo():
                    ps = projp.tile([128, QB], F32, tag="ps", name="ps")
                    if kind in ("q", "k"):
                        w_sb = wq_sb if kind == "q" else wk_sb
                        dst = qhT if kind == "q" else khT
                        for kc in range(KC):
                            nc.tensor.matmul(
                                ps[:],
                                w_sb[:, kc, m * 128 : (m + 1) * 128],
                                xt[kind][:, kc, :],
                                start=(kc == 0),
                                stop=(kc == KC - 1),
                            )
                        nc.vector.tensor_copy(
                            dst[:, m, n * QB : (n + 1) * QB], ps[:]
                        )
                    else:
                        for kc in range(KC):
                            nc.tensor.matmul(
                                ps[:, 0:DH],
                                xt["v"][:, kc, m * 128 : (m + 1) * 128],
                                wv_sb[:, kc, :],
                                start=(kc == 0),
                                stop=(kc == KC - 1),
                            )
                        tci = n * KPQ + m
                        vdst = vh[:, tci, :].rearrange(
                            "p (h c) -> p h c", c=HD + 1
                        )
                        nc.vector.tensor_copy(
                            vdst[:, :, 0:HD],
                            ps[:, 0:DH].rearrange("p (h d) -> p h d", d=HD),
                        )
                return clo

            def proj_groups(n, xt):
                return [
                    proj_group(n, xt, kind, m)
                    for kind in ("q", "k", "v")
                    for m in range(NHP)
                ]

            # ---- SDPA section for q-block qb: blocks (hp, qb), lag-2,
            # with proj/outproj/norm groups interleaved as PE filler work ----
            def emit_sdpa_section(qb, fillers):
                kcnt = (qb + 1) * KPQ
                q0 = qb * QB
                steps = [(hp, kc) for hp in range(NHP) for kc in range(kcnt)]
                state = {}

                def emit_s(i):
                    hp, kc = steps[i]
                    if hp not in state:
                        state[hp] = {"p": {}, "o": None}
                    stt = state[hp]
                    dj = kc - qb * KPQ
                    c0 = 128 * dj if dj > 0 else 0
                    spair = spool.tile([128, 2, QB], F32, tag="s", name="spair")
                    nc.tensor.matmul(
                        spair[:, 0, c0:QB],
                        khT[0:64, hp, kc * 128 : (kc + 1) * 128],
                        qhT[0:64, hp, q0 + c0 : q0 + QB],
                        start=True,
                        stop=True,
                    )
                    nc.tensor.matmul(
                        spair[:, 1, c0:QB],
                        khT[64:128, hp, kc * 128 : (kc + 1) * 128],
                        qhT[64:128, hp, q0 + c0 : q0 + QB],
                        start=True,
                        stop=True,
                    )
                    ppair = ppool.tile([128, 2, QB], BF16, tag="p", name="ppair")
                    nc.scalar.activation(
                        ppair[:, :, c0:QB], spair[:, :, c0:QB], EXP
                    )
                    if dj >= 0:
                        for h in range(2):
                            nc.vector.tensor_mul(
                                ppair[:, h, c0 : c0 + 128],
                                ppair[:, h, c0 : c0 + 128],
                                tri_bf[:],
                            )
                    stt["p"][kc] = ppair

                def emit_pv(i):
                    hp, kc = steps[i]
                    stt = state[hp]
                    dj = kc - qb * KPQ
                    c0 = 128 * dj if dj > 0 else 0
                    if kc == 0:
                        stt["o"] = (
                            opool.tile([65, QB], F32, tag="o", name="o_a"),
                            opool.tile([65, QB], F32, tag="o", name="o_b"),
                        )
                    ppair = stt["p"].pop(kc)
                    for h in range(2):
                        nc.tensor.matmul(
                            stt["o"][h][:, c0:QB],
                            vh[:, kc, (2 * hp + h) * (HD + 1) : (2 * hp + h + 1) * (HD + 1)],
                            ppair[:, h, c0:QB],
                            start=(kc == 0),
                            stop=(kc == kcnt - 1),
                        )
                    if kc == kcnt - 1:
                        # unnormalized o -> SBUF; denominator rows -> obden
                        stt = state.pop(hp)
                        oA, oB = stt["o"]
                        nc.vector.tensor_copy(
                            oT[0:64, hp, q0 : q0 + QB], oA[0:64, :]
                        )
                        nc.vector.tensor_copy(
                            obden[64:65, qb % 2, hp * 2, :], oA[64:65, :]
                        )
                        nc.vector.tensor_copy(
                            obden[0:65, qb % 2, hp * 2 + 1, :], oB[0:65, :]
                        )

                LAG = 2
                total = len(steps) + LAG
                fi = 0
                for i in range(total):
                    if i < len(steps):
                        emit_s(i)
                    if i - LAG >= 0:
                        emit_pv(i - LAG)
                    want = (i + 1) * len(fillers) // total
                    while fi < want:
                        fillers[fi]()
                        fi += 1
                while fi < len(fillers):
                    fillers[fi]()
                    fi += 1

            # ---- deferred normalization for section qb ----
            def emit_norm(qb):
                q0 = qb * QB
                den_t = projp.tile([128, QB], F32, tag="ps", name="den_t")
                for jl in range(8):              # jl = hp*2 + head
                    for c in range(4):
                        nc.tensor.matmul(
                            den_t[:, jl * 4 + c : jl * 4 + c + 1],
                            obden[64:65, qb % 2, jl, c * 128 : (c + 1) * 128],
                            ones64[64:65, 0:1],
                            start=(jl == 0 and c == 0),
                            stop=(jl == 7 and c == 3),
                            skip_group_check=True,
                        )
                den_rt = drpool.tile([128, 32], F32, tag="dr", name="den_rt")
                nc.vector.reciprocal(den_rt[:], den_t[:, 0:32])
                for jl in range(8):
                    hp, h = jl // 2, jl % 2
                    bc = projp.tile([128, QB], F32, tag="ps", name="bc")
                    for c in range(4):
                        rrep = rrpool.tile([128, 64], F16, tag="rr", name="rrep")
                        nc.vector.tensor_scalar_mul(
                            rrep[:],
                            ones64[:],
                            den_rt[:, jl * 4 + c : jl * 4 + c + 1],
                        )
                        nc.tensor.matmul(
                            bc[0:64, c * 128 : (c + 1) * 128],
                            rrep[:],
                            ident_sb[:],
                            start=(c == 0),
                            stop=(c == 3),
                            skip_group_check=True,
                        )
                    if h == 0:
                        dst = oT[0:64, hp, q0 : q0 + QB]
                        nc.vector.tensor_mul(dst, dst, bc[0:64, :])
                    else:
                        dst = obden[0:64, qb % 2, hp * 2 + 1, :]
                        nc.vector.tensor_mul(dst, dst, bc[0:64, :])
                        nc.sync.dma_start(
                            oT[64:128, hp, q0 : q0 + QB], dst
                        )

            # ---- main interleaved pipeline ----
            xt = emit_x_dma(0)
            for clo in proj_groups(0, xt):
                clo()
            for n in range(NT):
                fillers = []
                if n + 1 < NT:
                    xt = emit_x_dma(n + 1)
                    fillers += proj_groups(n + 1, xt)
                if n >= 1:
                    # norm MUST precede outproj of the same q-block (in-place
                    # normalization of oT vs program-order reads)
                    fillers.insert(
                        min(2, len(fillers)), lambda m=n - 1: emit_norm(m)
                    )
                    fillers += [outproj_group(n - 1, m) for m in range(D // 128)]
                emit_sdpa_section(n, fillers)
            emit_norm(NT - 1)
            for m in range(D // 128):
                outproj_group(NT - 1, m)()

    return nc


def build():
    nc = bacc.Bacc("TRN2", target_bir_lowering=False, debug=False)
    _emit(nc)
    nc.compile()
    return nc


_NC_CACHE = {}


def _get_nc():
    if "nc" not in _NC_CACHE:
        _NC_CACHE["nc"] = build()
    return _NC_CACHE["nc"]


def make_in_maps(q, k, v, Wq, Wk, Wv, Wo):
    konst_m = np.zeros((128, 256), dtype=np.float16)
    konst_m[:, 0:128] = np.triu(np.ones((128, 128), dtype=np.float16))
    konst_m[:, 128:256] = np.eye(128, dtype=np.float16)
    def xlay(xb):
        # x[b] [L, D] -> xT [D, L] -> [kc, 128, NT, QB] -> [128, NT, kc, QB]
        xT = xb.T.reshape(KC, 128, NT, QB)
        return np.ascontiguousarray(
            xT.transpose(1, 2, 0, 3)
        ).astype(np.float16)

    def wlay(wt):
        # wt [D, DH_like] -> [kc, 128, M] -> [128, kc, M]
        m = wt.shape[1]
        return np.ascontiguousarray(
            wt.reshape(KC, 128, m).transpose(1, 0, 2)
        ).astype(np.float16)

    qT = [xlay(np.asarray(q[b])) for b in range(B)]
    kT = [xlay(np.asarray(k[b])) for b in range(B)]
    vT = [xlay(np.asarray(v[b])) for b in range(B)]
    wq_s, wk_s, wv_s, wo_s = [], [], [], []
    for tp in range(TP):
        rows = slice(tp * DH, (tp + 1) * DH)
        wq_s.append(wlay(Wq[rows].T * SCALE))
        wk_s.append(wlay(Wk[rows].T))
        wv_s.append(wlay(Wv[rows].T))
        # wo [DH, D]: DH = 4 chunks of 128 -> [128, NHP, D]
        wo_s.append(np.ascontiguousarray(
            Wo[:, rows].T.reshape(NHP, 128, D).transpose(1, 0, 2)
        ).astype(np.float16))
    in_maps = []
    for c in range(N_CORES):
        b, tp = c // TP, c % TP
        in_maps.append(
            {
                "xq": qT[b],
                "xk": kT[b],
                "xv": vT[b],
                "wq": wq_s[tp],
                "wk": wk_s[tp],
                "wv": wv_s[tp],
                "wo": wo_s[tp],
                "konst": konst_m,
            }
        )
    return in_maps


def kernel(q, k, v, Wq, Wk, Wv, Wo, mask=None, trace=False):
    q = np.asarray(q, dtype=np.float32)
    k = np.asarray(k, dtype=np.float32)
    v = np.asarray(v, dtype=np.float32)
    nc = _get_nc()
    in_maps = make_in_maps(
        q, k, v,
        np.asarray(Wq, np.float32), np.asarray(Wk, np.float32),
        np.asarray(Wv, np.float32), np.asarray(Wo, np.float32),
    )
    res = run_bass_kernel_spmd(
        nc, in_maps, core_ids=list(range(N_CORES)), trace=trace
    )
    out = np.zeros((B, L, D), dtype=np.float32)
    for c in range(N_CORES):
        out[c // TP] += res.results[c]["outT"].T.astype(np.float32)
    if trace:
        return out, res
    return out
